# revision 5
# baseline (speedup 1.0000x reference)
"""MinGRU (parallel log-space scan) Trainium2 Bass kernel.

Problem (hardcoded):
    x:    [B=8, S=4096, D=1024] f32
    W_hg: [D=1024, 2*D=2048]    f32
    out:  [B=8, S=4096, D=1024] f32

    hg = x @ W_hg ; hidden, gate = split(hg)
    h_t = (1-z_t) * h_{t-1} + z_t * g(hidden_t),  z = sigmoid(gate),
    g(v) = v + 0.5 if v >= 0 else sigmoid(v)  ==  max(v + 0.5, sigmoid(v))

Sharding: data-parallel over batch, one batch row per NeuronCore (8 cores),
W_hg replicated.

The scan runs along the free dimension (channels on partitions), so the
device works entirely in the transposed layout hg^T/h^T = [channels, seq];
the host passes x pre-transposed per batch row and transposes the returned
h^T back.

v2 layout/perf notes:
  - All matmul operands are bf16 (host-cast): same 1 cyc/row PE rate as
    fp32r but half the DMA bytes and fast weight loads (FWL).
  - x^T and W are fully resident in SBUF (96 KiB/partition at bf16), so
    there are no input DMAs or pool-recycle waits in steady state.
  - DMA trigger count is minimized (one trigger per x chunk / W k-slice):
    one HWDGE trigger spreads across all 16 SDMA engines, and the serial
    ~0.6us per-trigger issue cost on the Sync queue was the reason the
    baseline's first matmul started at 17us.
  - Input triggers are split across the two HWDGE rings: w[k=0] + x chunks
    on the Scalar ring, w[k=1..7] on the Sync ring, so the first matmul
    only waits for ~1 MiB. Output stores go on the Sync ring.
  - W is host-shuffled to [d, (k, half, c)] so each k-slice (hidden+gate
    128-col blocks for all 8 j) is one contiguous-column trigger.
  - Chunks are [256, 512*7, 256]: small first chunk starts the PE sooner,
    small last chunk (plus 128-col scan/store splits for the last two k)
    shortens the serial pointwise tail after the final matmul.

Per-core pipeline per (chunk sc, channel tile k):
  fp32-PSUM bf16 matmuls hg^T[k] = sum_j W[j,k]^T x^T[j]
  -> ACT: a = sigmoid(-gate), sigh = sigmoid(hidden)      [PSUM -> SBUF]
  -> DVE: gh = (hidden + 0.5) max sigh ; bneg = (a - 1) * gh
  -> DVE: h = scan(a * h_prev) - bneg   (carry chained across chunks)
  -> DMA h^T tile straight to DRAM out^T.
"""

import numpy as np
import ml_dtypes

import concourse.bacc as bacc
import concourse.tile as tile
from concourse import mybir

B, S, D = 8, 4096, 1024
N_CORES = 8
P = 128  # partitions
N_DT = D // P  # 8 contraction (j) tiles
N_KT = D // P  # 8 output channel (k) tiles per half
CHUNKS = [256] + [512] * 7 + [256]
assert sum(CHUNKS) == S

BF16 = mybir.dt.bfloat16
F32 = mybir.dt.float32

_COMPILED = {}


def _build():
    nc = bacc.Bacc(
        "TRN2", target_bir_lowering=False, debug=False, num_devices=N_CORES
    )
    xt_d = nc.dram_tensor("xt", [D, S], BF16, kind="ExternalInput").ap()
    # host-shuffled: w[j*128+p, (k*2+half)*128+c] = W_hg[j*128+p, half*D+k*128+c]
    w_d = nc.dram_tensor("w", [D, 2 * D], BF16, kind="ExternalInput").ap()
    out_d = nc.dram_tensor("outT", [D, S], F32, kind="ExternalOutput").ap()

    AL = mybir.AluOpType
    SIG = mybir.ActivationFunctionType.Sigmoid

    with tile.TileContext(nc) as tc:
        with (
            tc.tile_pool(name="wpool", bufs=1) as wpool,
            tc.tile_pool(name="xpool", bufs=1) as x_pool,
            tc.tile_pool(name="pw", bufs=3) as pw_pool,
            tc.tile_pool(name="hp", bufs=3) as h_pool,
            tc.tile_pool(name="pshg", bufs=8, space="PSUM") as psum_hg,
        ):
            # SBUF weight tile, free layout (j, k, half, c): 16384 cols bf16
            wt = wpool.tile([P, N_DT * 2 * D], BF16, tag="w", name="wt")
            wt3 = wt.rearrange("p (j r) -> p j r", j=N_DT)
            w3 = w_d.rearrange("(j p) r -> p j r", p=P)

            def wload(eng, k):
                # one trigger: both 128-col half-blocks of output tile k,
                # for all 8 j (contiguous 256 cols in the shuffled layout)
                eng.dma_start(
                    wt3[:, :, k * 2 * P : (k + 1) * 2 * P],
                    w3[:, :, k * 2 * P : (k + 1) * 2 * P],
                )

            # x^T fully resident: one SBUF tile per chunk, free layout (j, c)
            xt3 = xt_d.rearrange("(j p) s -> p j s", p=P)
            xtiles = []
            starts = []
            s0 = 0
            for sc, C in enumerate(CHUNKS):
                t = x_pool.tile([P, N_DT * C], BF16, tag=f"xc{sc}", name=f"xc{sc}")
                xtiles.append(t)
                starts.append(s0)
                s0 += C

            def xload(sc):
                C, s0 = CHUNKS[sc], starts[sc]
                nc.scalar.dma_start(
                    xtiles[sc].rearrange("p (j c) -> p j c", j=N_DT),
                    xt3[:, :, s0 : s0 + C],
                )

            # critical path (Scalar ring): w[k=0], x chunk 0, then the rest
            # of x; w[k=1..7] in parallel on the Sync ring.
            wload(nc.scalar, 0)
            xload(0)
            xload(1)
            for k in range(1, N_KT):
                wload(nc.sync, k)
            for sc in range(2, len(CHUNKS)):
                xload(sc)

            def w_sb(j, k, half):
                off = ((j * N_KT + k) * 2 + half) * P
                return wt[:, off : off + P]

            prev_h = [None] * N_KT
            last = len(CHUNKS) - 1
            for sc, C in enumerate(CHUNKS):
                s0 = starts[sc]
                xt_sb = xtiles[sc]
                for k in range(N_KT):
                    ph = psum_hg.tile([P, C], F32, tag="ph")  # hidden
                    for j in range(N_DT):
                        nc.tensor.matmul(
                            ph[:],
                            w_sb(j, k, 0),
                            xt_sb[:, j * C : (j + 1) * C],
                            start=(j == 0),
                            stop=(j == N_DT - 1),
                        )
                    pg = psum_hg.tile([P, C], F32, tag="ph")  # gate
                    for j in range(N_DT):
                        nc.tensor.matmul(
                            pg[:],
                            w_sb(j, k, 1),
                            xt_sb[:, j * C : (j + 1) * C],
                            start=(j == 0),
                            stop=(j == N_DT - 1),
                        )
                    # a = sigmoid(-gate) = 1 - z
                    a_t = pw_pool.tile([P, C], F32, tag="a")
                    nc.scalar.activation(a_t[:], pg[:], SIG, scale=-1.0)
                    # sigh = sigmoid(hidden)
                    sigh = pw_pool.tile([P, C], F32, tag="sigh")
                    nc.scalar.activation(sigh[:], ph[:], SIG)
                    # g(hidden) = max(hidden + 0.5, sigmoid(hidden))
                    gh = pw_pool.tile([P, C], F32, tag="gh")
                    nc.vector.scalar_tensor_tensor(
                        gh[:], ph[:], 0.5, sigh[:], op0=AL.add, op1=AL.max
                    )
                    # bneg = (a - 1) * g = -(z * g)
                    bneg = pw_pool.tile([P, C], F32, tag="bneg")
                    nc.vector.scalar_tensor_tensor(
                        bneg[:], a_t[:], 1.0, gh[:], op0=AL.subtract, op1=AL.mult
                    )
                    # h_t = a_t * h_{t-1} - bneg_t  (linear recurrence)
                    h = h_pool.tile([P, C], F32, tag=f"h{k}")
                    init = 0.0 if prev_h[k] is None else prev_h[k][:, -1:]
                    if sc == last and k >= N_KT - 2:
                        # kernel tail: 128-col scan/store pieces so the
                        # stores overlap the remaining scans
                        H = C // 2
                        nc.vector.tensor_tensor_scan(
                            h[:, 0:H], a_t[:, 0:H], bneg[:, 0:H], init,
                            op0=AL.mult, op1=AL.subtract,
                        )
                        nc.sync.dma_start(
                            out_d[k * P : (k + 1) * P, s0 : s0 + H], h[:, 0:H]
                        )
                        nc.vector.tensor_tensor_scan(
                            h[:, H:C], a_t[:, H:C], bneg[:, H:C],
                            h[:, H - 1 : H], op0=AL.mult, op1=AL.subtract,
                        )
                        nc.sync.dma_start(
                            out_d[k * P : (k + 1) * P, s0 + H : s0 + C],
                            h[:, H:C],
                        )
                    else:
                        nc.vector.tensor_tensor_scan(
                            h[:], a_t[:], bneg[:], init,
                            op0=AL.mult, op1=AL.subtract,
                        )
                        nc.sync.dma_start(
                            out_d[k * P : (k + 1) * P, s0 : s0 + C], h[:]
                        )
                    prev_h[k] = h
    nc.compile()
    return nc


def _get_nc():
    if "nc" not in _COMPILED:
        _COMPILED["nc"] = _build()
    return _COMPILED["nc"]


def make_in_maps(x: np.ndarray, W_hg: np.ndarray):
    bf = ml_dtypes.bfloat16
    # shuffle W columns: [half, k, c] -> [k, half, c]
    w = np.asarray(W_hg, dtype=np.float32).reshape(D, 2, N_KT, P)
    w = np.ascontiguousarray(w.transpose(0, 2, 1, 3)).reshape(D, 2 * D)
    w = w.astype(bf)
    x = np.asarray(x, dtype=np.float32)
    return [
        {"xt": np.ascontiguousarray(x[b].T).astype(bf), "w": w}
        for b in range(N_CORES)
    ]


def kernel(x: np.ndarray, W_hg: np.ndarray) -> np.ndarray:
    from concourse.bass_utils import run_bass_kernel_spmd

    assert x.shape == (B, S, D) and W_hg.shape == (D, 2 * D)
    nc = _get_nc()
    in_maps = make_in_maps(x, W_hg)
    res = run_bass_kernel_spmd(nc, in_maps, list(range(N_CORES)))
    out = np.empty((B, S, D), dtype=np.float32)
    for b in range(N_CORES):
        out[b] = res.results[b]["outT"].T
    return out


# revision 9
# speedup vs baseline: 1.0688x; 1.0688x over previous
"""MinGRU (parallel log-space scan) Trainium2 Bass kernel.

Problem (hardcoded):
    x:    [B=8, S=4096, D=1024] f32
    W_hg: [D=1024, 2*D=2048]    f32
    out:  [B=8, S=4096, D=1024] f32

    hg = x @ W_hg ; hidden, gate = split(hg)
    h_t = (1-z_t) * h_{t-1} + z_t * g(hidden_t),  z = sigmoid(gate),
    g(v) = v + 0.5 if v >= 0 else sigmoid(v)  ==  max(v + 0.5, sigmoid(v))

Sharding: data-parallel over batch, one batch row per NeuronCore (8 cores),
W_hg replicated.

The scan runs along the free dimension (channels on partitions), so the
device works entirely in the transposed layout hg^T/h^T = [channels, seq];
the host passes x pre-transposed per batch row and transposes the returned
h^T back.

v2 layout/perf notes:
  - All matmul operands are bf16 (host-cast): same 1 cyc/row PE rate as
    fp32r but half the DMA bytes and fast weight loads (FWL).
  - x^T and W are fully resident in SBUF (96 KiB/partition at bf16), so
    there are no input DMAs or pool-recycle waits in steady state.
  - DMA trigger count is minimized (one trigger per x chunk / W k-slice):
    one HWDGE trigger spreads across all 16 SDMA engines, and the serial
    ~0.6us per-trigger issue cost on the Sync queue was the reason the
    baseline's first matmul started at 17us.
  - Input triggers are split across the two HWDGE rings: w[k=0] + x chunks
    on the Scalar ring, w[k=1..7] on the Sync ring, so the first matmul
    only waits for ~1 MiB. Output stores go on the Sync ring.
  - W is host-shuffled to [d, (k, half, c)] so each k-slice (hidden+gate
    128-col blocks for all 8 j) is one contiguous-column trigger.
  - Chunks are [256, 512*7, 256]: small first chunk starts the PE sooner,
    small last chunk (plus 128-col scan/store splits for the last two k)
    shortens the serial pointwise tail after the final matmul.

Per-core pipeline per (chunk sc, channel tile k):
  fp32-PSUM bf16 matmuls hg^T[k] = sum_j W[j,k]^T x^T[j]
  -> ACT: a = sigmoid(-gate), sigh = sigmoid(hidden)      [PSUM -> SBUF]
  -> DVE: gh = (hidden + 0.5) max sigh ; bneg = (a - 1) * gh
  -> DVE: h = scan(a * h_prev) - bneg   (carry chained across chunks)
  -> DMA h^T tile straight to DRAM out^T.
"""

import numpy as np
import ml_dtypes

import concourse.bacc as bacc
import concourse.tile as tile
from concourse import mybir

B, S, D = 8, 4096, 1024
N_CORES = 8
P = 128  # partitions
N_DT = D // P  # 8 contraction (j) tiles
N_KT = D // P  # 8 output channel (k) tiles per half
CHUNKS = [256] + [512] * 7 + [256]
assert sum(CHUNKS) == S

BF16 = mybir.dt.bfloat16
F32 = mybir.dt.float32

_COMPILED = {}


def _build():
    nc = bacc.Bacc(
        "TRN2", target_bir_lowering=False, debug=False, num_devices=N_CORES
    )
    # Both inputs are host-shuffled into the exact SBUF layout so every load
    # is a 2D copy of 128 DRAM rows with multi-KB contiguous lines: HWDGE
    # descriptor generation costs ~5ns per row, so tall/skinny APs (1024
    # rows) would serialize the preload and starve the PE.
    #   xt[p, 8*s0 + j*C + c] = x[s0+c, j*128+p]   (chunk-major)
    #   w[p, ((k*2+half)*8+j)*128+c] = W_hg[j*128+p, half*D+k*128+c]
    xt_d = nc.dram_tensor("xt", [P, N_DT * S], BF16, kind="ExternalInput").ap()
    w_d = nc.dram_tensor("w", [P, N_DT * 2 * D], BF16, kind="ExternalInput").ap()
    out_d = nc.dram_tensor("outT", [D, S], F32, kind="ExternalOutput").ap()

    AL = mybir.AluOpType
    SIG = mybir.ActivationFunctionType.Sigmoid

    with tile.TileContext(nc) as tc:
        with (
            tc.tile_pool(name="wpool", bufs=1) as wpool,
            tc.tile_pool(name="xpool", bufs=1) as x_pool,
            tc.tile_pool(name="pw", bufs=3) as pw_pool,
            tc.tile_pool(name="hp", bufs=3) as h_pool,
            tc.tile_pool(name="pshg", bufs=8, space="PSUM") as psum_hg,
        ):
            # SBUF weight tile, free layout (k, half, j, c): 16384 cols bf16
            wt = wpool.tile([P, N_DT * 2 * D], BF16, tag="w", name="wt")
            KB = 2 * N_DT * P  # 2048 cols per k

            def wload(eng, k):
                # one trigger: all 16 [128,128] blocks of output tile k
                # (contiguous 2048 cols / 4KB lines in the shuffled layout)
                eng.dma_start(
                    wt[:, k * KB : (k + 1) * KB], w_d[:, k * KB : (k + 1) * KB]
                )

            # x^T fully resident: one SBUF tile per chunk, free layout (j, c)
            xtiles = []
            starts = []
            s0 = 0
            for sc, C in enumerate(CHUNKS):
                t = x_pool.tile([P, N_DT * C], BF16, tag=f"xc{sc}", name=f"xc{sc}")
                xtiles.append(t)
                starts.append(s0)
                s0 += C

            def xload(sc):
                C, s0 = CHUNKS[sc], starts[sc]
                nc.scalar.dma_start(
                    xtiles[sc][:], xt_d[:, N_DT * s0 : N_DT * (s0 + C)]
                )

            # critical path (Scalar ring): w[k=0], x chunk 0, then the rest
            # of x; w[k=1..7] in parallel on the Sync ring.
            wload(nc.scalar, 0)
            xload(0)
            xload(1)
            for k in range(1, N_KT):
                wload(nc.sync, k)
            for sc in range(2, len(CHUNKS)):
                xload(sc)

            def w_sb(j, k, half):
                off = ((k * 2 + half) * N_DT + j) * P
                return wt[:, off : off + P]

            prev_h = [None] * N_KT
            last = len(CHUNKS) - 1
            for sc, C in enumerate(CHUNKS):
                s0 = starts[sc]
                xt_sb = xtiles[sc]
                for k in range(N_KT):
                    ph = psum_hg.tile([P, C], F32, tag="ph")  # hidden
                    for j in range(N_DT):
                        nc.tensor.matmul(
                            ph[:],
                            w_sb(j, k, 0),
                            xt_sb[:, j * C : (j + 1) * C],
                            start=(j == 0),
                            stop=(j == N_DT - 1),
                        )
                    pg = psum_hg.tile([P, C], F32, tag="ph")  # gate
                    for j in range(N_DT):
                        nc.tensor.matmul(
                            pg[:],
                            w_sb(j, k, 1),
                            xt_sb[:, j * C : (j + 1) * C],
                            start=(j == 0),
                            stop=(j == N_DT - 1),
                        )
                    # a = sigmoid(-gate) = 1 - z
                    a_t = pw_pool.tile([P, C], F32, tag="a")
                    nc.scalar.activation(a_t[:], pg[:], SIG, scale=-1.0)
                    # sigh = sigmoid(hidden)
                    sigh = pw_pool.tile([P, C], F32, tag="sigh")
                    nc.scalar.activation(sigh[:], ph[:], SIG)
                    # g(hidden) = max(hidden + 0.5, sigmoid(hidden))
                    gh = pw_pool.tile([P, C], F32, tag="gh")
                    nc.vector.scalar_tensor_tensor(
                        gh[:], ph[:], 0.5, sigh[:], op0=AL.add, op1=AL.max
                    )
                    # bneg = (a - 1) * g = -(z * g)
                    bneg = pw_pool.tile([P, C], F32, tag="bneg")
                    nc.vector.scalar_tensor_tensor(
                        bneg[:], a_t[:], 1.0, gh[:], op0=AL.subtract, op1=AL.mult
                    )
                    # h_t = a_t * h_{t-1} - bneg_t  (linear recurrence)
                    h = h_pool.tile([P, C], F32, tag=f"h{k}")
                    init = 0.0 if prev_h[k] is None else prev_h[k][:, -1:]
                    if sc == last and k >= N_KT - 2:
                        # kernel tail: 128-col scan/store pieces so the
                        # stores overlap the remaining scans
                        H = C // 2
                        nc.vector.tensor_tensor_scan(
                            h[:, 0:H], a_t[:, 0:H], bneg[:, 0:H], init,
                            op0=AL.mult, op1=AL.subtract,
                        )
                        nc.sync.dma_start(
                            out_d[k * P : (k + 1) * P, s0 : s0 + H], h[:, 0:H]
                        )
                        nc.vector.tensor_tensor_scan(
                            h[:, H:C], a_t[:, H:C], bneg[:, H:C],
                            h[:, H - 1 : H], op0=AL.mult, op1=AL.subtract,
                        )
                        nc.sync.dma_start(
                            out_d[k * P : (k + 1) * P, s0 + H : s0 + C],
                            h[:, H:C],
                        )
                    else:
                        nc.vector.tensor_tensor_scan(
                            h[:], a_t[:], bneg[:], init,
                            op0=AL.mult, op1=AL.subtract,
                        )
                        nc.sync.dma_start(
                            out_d[k * P : (k + 1) * P, s0 : s0 + C], h[:]
                        )
                    prev_h[k] = h
    nc.compile()
    return nc


def _get_nc():
    if "nc" not in _COMPILED:
        _COMPILED["nc"] = _build()
    return _COMPILED["nc"]


def make_in_maps(x: np.ndarray, W_hg: np.ndarray):
    bf = ml_dtypes.bfloat16
    # W_hg [j*128+p, half*D+k*128+c] -> w[p, ((k*2+half)*8+j)*128+c]
    w = np.asarray(W_hg, dtype=np.float32).reshape(N_DT, P, 2, N_KT, P)
    w = w.transpose(1, 3, 2, 0, 4).reshape(P, N_DT * 2 * D)
    w = np.ascontiguousarray(w).astype(bf)
    x = np.asarray(x, dtype=np.float32)
    in_maps = []
    for b in range(N_CORES):
        xb = x[b].astype(bf)  # [S, D]
        blocks = []
        s0 = 0
        for C in CHUNKS:
            # x[s0+c, j*128+p] -> [p, j*C+c]
            blk = xb[s0 : s0 + C].T.reshape(N_DT, P, C)
            blocks.append(blk.transpose(1, 0, 2).reshape(P, N_DT * C))
            s0 += C
        xt = np.ascontiguousarray(np.concatenate(blocks, axis=1))
        in_maps.append({"xt": xt, "w": w})
    return in_maps


def kernel(x: np.ndarray, W_hg: np.ndarray) -> np.ndarray:
    from concourse.bass_utils import run_bass_kernel_spmd

    assert x.shape == (B, S, D) and W_hg.shape == (D, 2 * D)
    nc = _get_nc()
    in_maps = make_in_maps(x, W_hg)
    res = run_bass_kernel_spmd(nc, in_maps, list(range(N_CORES)))
    out = np.empty((B, S, D), dtype=np.float32)
    for b in range(N_CORES):
        out[b] = res.results[b]["outT"].T
    return out
